# revision 14
# baseline (speedup 1.0000x reference)
"""Multi-head causal attention (LLaMA RoPE), head-parallel sharding on 8 trn2 cores.

Core c = (batch b=c//2, head-group g=c%2). Each core computes Q/K/V projections
and attention for its 8 heads over the FULL 1024 rows of its batch (zero
duplication). All matmuls run fp32r: on TRN2 fp32r streams 1 row/cycle at
free-size >= 256 while bf16 takes 2 cycles/row, so fp32r is the fast path
everywhere including the O-projection (the baseline's bf16 o_pass paid 2x).

O-projection is split local/remote: each pair's attention output (ot) stays
SBUF-resident and its y contribution accumulates with no collective
dependency; only the peer head-group's ot crosses the wire (pair AllGather
via DRAM), then a conditional DMA (cond = host-supplied group flag, keeping
the SPMD program symmetric) pulls just the peer rows back to SBUF for the
remote half. Two HWDGE queues: sync carries input streams (x, weights,
tables), scalar/Act carries exchange+output streams (ot, sot, y), so
collectives never queue behind weight prefetches.

Scores are computed transposed ST[k,q]; softmax skips max-subtraction;
denominator via ones-matmul accumulated alongside PV; causal mask applied
post-exp as a binary multiply on diagonal 128x128 tiles (widened to 256 on
the narrow tips so fp32r stays on its fast path). Attention issue order is
software-pipelined (2-deep ST lookahead) so the PE rides the Act exp chain
instead of waiting on it. x and pair-0 weights stream in fine-grained
interleaved chunks so the PE starts ~2us in.
"""

import math
import sys

import numpy as np

sys.path.insert(0, "/opt/trn_rl_repo")

B, S, DIM, H = 4, 1024, 2048, 16
HD = DIM // H  # 128
KC = DIM // 128  # 16 contraction chunks
HPC = H // 2  # 8 heads per core
NP = HPC // 2  # 4 head-pairs per core
EHALF = DIM // 2  # 1024 output cols per core
SCALE = 1.0 / math.sqrt(HD)
N_CORES = 8
PAIRS = [[0, 1], [2, 3], [4, 5], [6, 7]]

_cache = {}


def _build_nc():
    import concourse.mybir as mybir
    import concourse.tile as tile
    from concourse import bacc

    F32R = mybir.dt.float32r
    F32 = mybir.dt.float32
    BF16 = mybir.dt.bfloat16
    U32 = mybir.dt.uint32

    nc = bacc.Bacc("TRN2", target_bir_lowering=False, debug=False,
                   num_devices=N_CORES)

    x_in = nc.dram_tensor("x_pre", [2, 128, KC, 512], F32R, kind="ExternalInput")
    wq_in = nc.dram_tensor("wq_pre", [HPC, 128, KC, 128], F32R, kind="ExternalInput")
    wk_in = nc.dram_tensor("wk_pre", [HPC, 128, KC, 128], F32R, kind="ExternalInput")
    wv_in = nc.dram_tensor("wv_pre", [NP, 128, KC, 256], F32R, kind="ExternalInput")
    wol_in = nc.dram_tensor("wol_pre", [NP, 128, 2, EHALF], F32R, kind="ExternalInput")
    wor_in = nc.dram_tensor("wor_pre", [NP, 128, 2, EHALF], F32R, kind="ExternalInput")
    bq_in = nc.dram_tensor("bq_p", [128, HPC, 1], F32, kind="ExternalInput")
    bk_in = nc.dram_tensor("bk_p", [128, HPC, 1], F32, kind="ExternalInput")
    bv_in = nc.dram_tensor("bv_p", [128, NP, 256], BF16, kind="ExternalInput")
    csk_in = nc.dram_tensor("csk2", [128, S], BF16, kind="ExternalInput")
    ssk_in = nc.dram_tensor("ssk2", [128, S], BF16, kind="ExternalInput")
    triw_in = nc.dram_tensor("triw", [128, 256], BF16, kind="ExternalInput")
    ones_in = nc.dram_tensor("ones128", [128, 128], F32R, kind="ExternalInput")
    sel_lo_in = nc.dram_tensor("sel_lo", [1, 1], U32, kind="ExternalInput")
    sel_hi_in = nc.dram_tensor("sel_hi", [1, 1], U32, kind="ExternalInput")
    y_out = nc.dram_tensor("y", [S, EHALF], F32, kind="ExternalOutput")

    with tile.TileContext(nc) as tc:
        with (
            tc.tile_pool(name="consts", bufs=1) as consts,
            tc.tile_pool(name="xpool", bufs=1) as xpool,
            tc.tile_pool(name="wpool", bufs=1) as wpool,
            tc.tile_pool(name="qkv", bufs=1) as qkv,
            tc.tile_pool(name="rope", bufs=2) as rope,
            tc.tile_pool(name="ptp", bufs=2) as ptp,
            tc.tile_pool(name="otsp", bufs=1) as otsp,
            tc.tile_pool(name="sotp", bufs=1) as sotp,
            tc.tile_pool(name="yap", bufs=1) as yap,
            tc.tile_pool(name="ysp", bufs=2) as ysp,
            tc.tile_pool(name="dram", bufs=1, space="DRAM") as dram,
            tc.tile_pool(name="ps_qk", bufs=2, space="PSUM") as ps_qk,
            tc.tile_pool(name="ps_st", bufs=2, space="PSUM") as ps_st,
            tc.tile_pool(name="ps_acc", bufs=4, space="PSUM") as ps_acc,
        ):
            # group-select flags for the conditional peer-block DMA
            r_lo = nc.scalar.alloc_register("sel_lo_r")
            nc.scalar.reg_load(r_lo, sel_lo_in[0:1, 0:1])
            sel_lo = nc.scalar.snap(r_lo, donate=True, min_val=0, max_val=1)
            r_hi = nc.scalar.alloc_register("sel_hi_r")
            nc.scalar.reg_load(r_hi, sel_hi_in[0:1, 0:1])
            sel_hi = nc.scalar.snap(r_hi, donate=True, min_val=0, max_val=1)

            # ---- startup: interleaved x chunks + pair-0 weights on sync q ----
            bq_sb = consts.tile([128, HPC, 1], F32, tag="bq")
            bk_sb = consts.tile([128, HPC, 1], F32, tag="bk")
            nc.sync.dma_start(bq_sb[:], bq_in[:])
            nc.sync.dma_start(bk_sb[:], bk_in[:])

            x_sb = xpool.tile([128, KC, S], F32R, tag="x", name="x_sb")

            def x_dma(h, kcs):
                for kc in kcs:
                    nc.sync.dma_start(x_sb[:, kc, h * 512:h * 512 + 512],
                                      x_in[h, :, kc, :])

            wq_sb = {}
            wk_sb = {}

            def load_qk(p, lh):
                wq_sb[(p, lh)] = wpool.tile([128, KC, 128], F32R,
                                            tag=f"wq{lh}", name="wq_sb")
                wk_sb[(p, lh)] = wpool.tile([128, KC, 128], F32R,
                                            tag=f"wk{lh}", name="wk_sb")
                nc.sync.dma_start(wq_sb[(p, lh)][:], wq_in[2 * p + lh])
                nc.sync.dma_start(wk_sb[(p, lh)][:], wk_in[2 * p + lh])

            wv_sb = {}
            wol_sb = {}
            wor_sb = {}

            def load_v(p):
                wv_sb[p] = wpool.tile([128, KC, 256], F32R, tag="wv",
                                      name="wv_sb")
                nc.sync.dma_start(wv_sb[p][:], wv_in[p])

            def load_wo(p):
                wol_sb[p] = wpool.tile([128, 2, EHALF], F32R, tag="wol",
                                       name="wol_sb")
                wor_sb[p] = wpool.tile([128, 2, EHALF], F32R, tag="wor",
                                       name="wor_sb")
                nc.sync.dma_start(wol_sb[p][:], wol_in[p])
                nc.sync.dma_start(wor_sb[p][:], wor_in[p])

            load_qk(0, 0)
            x_dma(0, range(0, 8))
            load_qk(0, 1)
            x_dma(0, range(8, 16))
            csk_sb = consts.tile([128, S], BF16, tag="cs", name="csk_sb")
            ssk_sb = consts.tile([128, S], BF16, tag="ss", name="ssk_sb")
            nc.sync.dma_start(csk_sb[:], csk_in[:])
            nc.sync.dma_start(ssk_sb[:], ssk_in[:])
            load_v(0)
            x_dma(1, range(0, 8))
            bv_sb = consts.tile([128, NP, 256], BF16, tag="bv")
            nc.sync.dma_start(bv_sb[:], bv_in[:])
            triw_sb = consts.tile([128, 256], BF16, tag="triw")
            nc.sync.dma_start(triw_sb[:], triw_in[:])
            ones_sb = consts.tile([128, 128], F32R, tag="ones")
            nc.sync.dma_start(ones_sb[:], ones_in[:])
            x_dma(1, range(8, 16))
            load_wo(0)

            # ---- DRAM tiles for the ot exchange ----
            ag_ins = []
            ag_outs = []
            for p in range(NP - 1):
                ag_ins.append(dram.tile([256, S], F32R, name=f"agi{p}"))
                ag_outs.append(dram.tile([4, 128, S], F32R, name=f"ago{p}"))
            ag3_ins = [dram.tile([256, 512], F32R, name=f"agi3q{qc}")
                       for qc in range(2)]
            ag3_outs = [dram.tile([4, 128, 512], F32R, name=f"ago3q{qc}")
                        for qc in range(2)]

            y_acc = yap.tile([128, 8, 2, 512], BF16, name="y_acc")
            ots = {}  # (p, lh, qc) -> SBUF tile [128, 512] f32r
            sotr = {}  # p -> [128, 2, S] (p<3) / (3, qc) -> [128, 2, 512]

            def rope_chunk(pm, b_ap, cs_ap, ss_ap, dst_ap):
                """dst = rope(pm + bias); all [128, 512]."""
                tmp = rope.tile([128, 512], BF16, tag="tmp", name="tmp")
                nc.scalar.activation(
                    tmp[:], pm, mybir.ActivationFunctionType.Identity, bias=b_ap
                )
                tsw = rope.tile([128, 512], BF16, tag="tsw", name="tsw")
                nc.sync.dma_start(tsw[0:64, :], tmp[64:128, :])
                nc.sync.dma_start(tsw[64:128, :], tmp[0:64, :])
                nc.vector.tensor_mul(tmp[:], tmp[:], cs_ap)
                nc.vector.tensor_mul(tsw[:], tsw[:], ss_ap)
                nc.vector.tensor_add(dst_ap, tmp[:], tsw[:])

            def qk_proj(p):
                q_sb = [qkv.tile([128, S], F32R, tag=f"q{lh}", name="q_sb")
                        for lh in range(2)]
                k_sb = [qkv.tile([128, S], F32R, tag=f"k{lh}", name="k_sb")
                        for lh in range(2)]
                for lh in range(2):
                    h = 2 * p + lh
                    # c-major so the startup pair consumes x in arrival order
                    for c in range(2):
                        ccols = slice(c * 512, c * 512 + 512)
                        for dst, w_sb, b_sb in (
                            (q_sb[lh], wq_sb[(p, lh)], bq_sb),
                            (k_sb[lh], wk_sb[(p, lh)], bk_sb),
                        ):
                            pm = ps_qk.tile([128, 512], F32, tag="qk", name="pm")
                            for kc in range(KC):
                                nc.tensor.matmul(pm[:], w_sb[:, kc, :],
                                                 x_sb[:, kc, ccols],
                                                 start=(kc == 0),
                                                 stop=(kc == KC - 1))
                            rope_chunk(pm[:], b_sb[:, h, :], csk_sb[:, ccols],
                                       ssk_sb[:, ccols], dst[:, ccols])
                return q_sb, k_sb

            def v_proj(p):
                v_sb = qkv.tile([128, 8, 256], F32R, tag="v", name="v_sb")
                for st in range(8):
                    scols = slice(st * 128, st * 128 + 128)
                    vp = ps_qk.tile([128, 512], F32, tag="qk", name="vp")
                    for kc in range(KC):
                        nc.tensor.matmul(vp[:, 0:256], x_sb[:, kc, scols],
                                         wv_sb[p][:, kc, :],
                                         start=(kc == 0), stop=(kc == KC - 1))
                    nc.vector.tensor_add(v_sb[:, st, :], vp[:, 0:256],
                                         bv_sb[:, p, :])
                return v_sb

            def attn_block(p, lh, qc, q_sb, k_sb, v_sb):
                """One (lh, qc) attention block, software-pipelined issue.

                Narrow 128-wide tips are widened to 256 (fp32r fast path);
                the extra columns are fully-masked garbage zeroed by the
                widened mask, harmless to the l/o accumulations.
                """
                infos = []
                kts = range(4) if qc == 0 else range(8)
                for kt in kts:
                    if qc == 0:
                        qv = slice(kt * 128, 512)
                        dg = slice(kt * 128, kt * 128 + 128)
                    else:
                        qv = slice((kt - 4) * 128, 512) if kt >= 4 \
                            else slice(0, 512)
                        dg = slice((kt - 4) * 128, (kt - 4) * 128 + 128) \
                            if kt >= 4 else None
                    wide = qv.stop - qv.start == 128
                    if wide:
                        qv = slice(qv.start - 128, 512)
                        dg = qv
                    infos.append((kt, qv, dg, wide))
                n = len(infos)
                vcols = slice(lh * 128, lh * 128 + 128)
                l_ps = ps_acc.tile([128, 512], F32, tag="acc", name="l_ps")
                o_ps = ps_acc.tile([128, 512], F32, tag="acc", name="o_ps")
                PIPE = 2
                pts = {}
                for i in range(n + PIPE):
                    if i < n:
                        kt, qv, dg, wide = infos[i]
                        qg = slice(qc * 512 + qv.start, qc * 512 + 512)
                        st_ps = ps_st.tile([128, 512], F32, tag="st",
                                           name="st_ps")
                        nc.tensor.matmul(st_ps[:, qv],
                                         k_sb[lh][:, kt * 128:kt * 128 + 128],
                                         q_sb[lh][:, qg], start=True, stop=True)
                        pt = ptp.tile([128, 512], F32R, tag="pt", name="pt")
                        nc.scalar.activation(pt[:, qv], st_ps[:, qv],
                                             mybir.ActivationFunctionType.Exp,
                                             scale=SCALE)
                        if dg is not None:
                            msk = triw_sb[:] if wide else triw_sb[:, 128:256]
                            nc.vector.tensor_mul(pt[:, dg], pt[:, dg], msk)
                        pts[i] = (pt, qv)
                    j = i - PIPE
                    if 0 <= j < n:
                        pt, qv = pts.pop(j)
                        ktj = infos[j][0]
                        first, last = (j == 0), (j == n - 1)
                        nc.tensor.matmul(l_ps[:, qv], ones_sb[:], pt[:, qv],
                                         start=first, stop=last)
                        nc.tensor.matmul(o_ps[:, qv], v_sb[:, ktj, vcols],
                                         pt[:, qv], start=first, stop=last)
                rl = ysp.tile([128, 512], F32, tag="rl", bufs=1, name="rl")
                nc.vector.reciprocal_approx_fast(rl[:], l_ps[:])
                ot = otsp.tile([128, 512], F32R, tag=f"ot{lh}{qc}", name="ot")
                nc.vector.tensor_mul(ot[:], o_ps[:], rl[:])
                ots[(p, lh, qc)] = ot
                # exchange input on the Act queue
                if p < 3:
                    nc.scalar.dma_start(
                        ag_ins[p][lh * 128:lh * 128 + 128,
                                  qc * 512:qc * 512 + 512], ot[:])
                else:
                    nc.scalar.dma_start(
                        ag3_ins[qc][lh * 128:lh * 128 + 128, :], ot[:])

            def fetch_remote(p, qc=None):
                """Conditional DMA of the peer head-group's ot rows to SBUF."""
                if p < 3:
                    w, tag = S, "sotr"
                    src = ag_outs[p].rearrange("a r c -> r a c")
                else:
                    w, tag = 512, "sotr3"
                    src = ag3_outs[qc].rearrange("a r c -> r a c")
                t = sotp.tile([128, 2, w], F32R, tag=tag, name="sotr_t")
                nc.scalar.dma_start(t[:], src[:, 0:2, :], cond=sel_lo)
                nc.scalar.dma_start(t[:], src[:, 2:4, :], cond=sel_hi)
                sotr[p if p < 3 else (3, qc)] = t

            def o_comb(p):
                """Accumulate local pair-p + remote pair-(p-1) into y_acc."""
                for qt in range(8):
                    qc, col = qt // 4, (qt % 4) * 128
                    for eb in range(2):
                        ecols = slice(eb * 512, eb * 512 + 512)
                        y_ps = ps_qk.tile([128, 512], F32, tag="qk",
                                          name="y_ps")
                        nmm = 2 if p == 0 else 4
                        mm = 0
                        for a in range(2):
                            mm += 1
                            nc.tensor.matmul(
                                y_ps[:], ots[(p, a, qc)][:, col:col + 128],
                                wol_sb[p][:, a, ecols],
                                start=(mm == 1), stop=(mm == nmm))
                        if p > 0:
                            for a in range(2):
                                mm += 1
                                nc.tensor.matmul(
                                    y_ps[:],
                                    sotr[p - 1][:, a, qt * 128:qt * 128 + 128],
                                    wor_sb[p - 1][:, a, ecols],
                                    start=False, stop=(mm == nmm))
                        ya = y_acc[:, qt, eb, :]
                        if p == 0:
                            nc.vector.tensor_copy(ya, y_ps[:])
                        else:
                            nc.vector.tensor_add(ya, ya, y_ps[:])

            def o_final(qts, sot3):
                """Remote pair-3 chunks + final add and store."""
                for qt in qts:
                    col = (qt % 4) * 128
                    qcols = slice(qt * 128, qt * 128 + 128)
                    for eb in range(2):
                        ecols = slice(eb * 512, eb * 512 + 512)
                        y_ps = ps_qk.tile([128, 512], F32, tag="qk",
                                          name="y_ps")
                        for a in range(2):
                            nc.tensor.matmul(
                                y_ps[:], sot3[:, a, col:col + 128],
                                wor_sb[3][:, a, ecols],
                                start=(a == 0), stop=(a == 1))
                        y_sb = ysp.tile([128, 512], F32, tag="ysb",
                                        bufs=1, name="y_sb")
                        nc.vector.tensor_add(y_sb[:], y_acc[:, qt, eb, :],
                                             y_ps[:])
                        nc.scalar.dma_start(y_out[qcols, ecols], y_sb[:])

            # ================= main pair loop =================
            q_sb = k_sb = v_sb = None
            for p in range(NP):
                if p == 0:
                    q_sb, k_sb = qk_proj(0)
                    v_sb = v_proj(0)
                if p + 1 < NP:
                    load_qk(p + 1, 0)
                    load_qk(p + 1, 1)
                    load_v(p + 1)
                if p > 0:
                    load_wo(p)

                # ---- attention ----
                lq = [(lh, qc) for lh in range(2) for qc in range(2)] \
                    if p < 3 else [(lh, qc) for qc in range(2) for lh in range(2)]
                for bi, (lh, qc) in enumerate(lq):
                    attn_block(p, lh, qc, q_sb, k_sb, v_sb)
                    # fetch the previous pair's peer rows mid-attention: its
                    # AllGather finished long ago, so the act queue never
                    # blocks, and the data lands well before o_comb(p)
                    if bi == 0 and p > 0:
                        fetch_remote(p - 1)
                    if p == 3 and lh == 1:
                        nc.gpsimd.collective_compute(
                            "AllGather", mybir.AluOpType.bypass,
                            replica_groups=PAIRS,
                            ins=[ag3_ins[qc].opt()],
                            outs=[ag3_outs[qc].opt()],
                        )
                if p < 3:
                    nc.gpsimd.collective_compute(
                        "AllGather", mybir.AluOpType.bypass, replica_groups=PAIRS,
                        ins=[ag_ins[p].opt()], outs=[ag_outs[p].opt()],
                    )

                # ---- next proj, then o accumulation, then V ----
                if p + 1 < NP:
                    nq, nk = qk_proj(p + 1)
                    o_comb(p)
                    nv = v_proj(p + 1)
                    q_sb, k_sb, v_sb = nq, nk, nv
                else:
                    o_comb(3)
                    # fetches sit after o_comb so the act queue never blocks
                    # on AG3 waits while attention exps are still pending
                    fetch_remote(3, 0)
                    o_final(range(0, 4), sotr[(3, 0)])
                    fetch_remote(3, 1)
                    o_final(range(4, 8), sotr[(3, 1)])
    nc.compile()
    return nc


def _get_nc():
    if "nc" not in _cache:
        _cache["nc"] = _build_nc()
    return _cache["nc"]


def _head_perm():
    p = []
    for h in range(H):
        base = h * HD
        p += [base + 2 * j for j in range(HD // 2)]
        p += [base + 2 * j + 1 for j in range(HD // 2)]
    return np.array(p)


def _pack_thin(wT):
    # [2048(k), 2048(d)] -> [H, 128(p), KC, 128(d)]
    return np.ascontiguousarray(
        wT.reshape(KC, 128, H, 128).transpose(2, 1, 0, 3)
    )


def _pack_x(xb):
    # [rows 1024, 2048] -> [2(col-half), 128(p), KC, 512] (contiguous halves)
    xt = xb.T.reshape(KC, 128, 2, 512)
    return np.ascontiguousarray(xt.transpose(2, 1, 0, 3))


def _prep_inputs(inputs):
    import ml_dtypes

    BF = ml_dtypes.bfloat16
    x = np.asarray(inputs["x"], np.float32)
    freqs_cos = np.asarray(inputs["freqs_cos"], np.float32)
    freqs_sin = np.asarray(inputs["freqs_sin"], np.float32)
    mask = np.asarray(inputs["mask"], np.float32)
    wq = np.asarray(inputs["wq"], np.float32)
    bq = np.asarray(inputs["bq"], np.float32)
    wk = np.asarray(inputs["wk"], np.float32)
    bk = np.asarray(inputs["bk"], np.float32)
    wv = np.asarray(inputs["wv"], np.float32)
    bv = np.asarray(inputs["bv"], np.float32)
    wo = np.asarray(inputs["wo"], np.float32)
    start_pos = int(np.asarray(inputs.get("start_pos", 0)))

    perm = _head_perm()
    wq_all = _pack_thin(np.ascontiguousarray(wq[perm].T))  # [H,128,KC,128]
    wk_all = _pack_thin(np.ascontiguousarray(wk[perm].T))
    bq_all = bq[perm].reshape(H, 128)  # [H, 128]
    bk_all = bk[perm].reshape(H, 128)

    wvT = np.ascontiguousarray(wv.T)  # [in 2048, out 2048]
    woT = np.ascontiguousarray(wo.T)  # [d 2048, e 2048]

    cosT = freqs_cos[start_pos:start_pos + S].T.astype(np.float32)  # [64, S]
    sinT = freqs_sin[start_pos:start_pos + S].T.astype(np.float32)
    csk2 = np.ascontiguousarray(np.vstack([cosT, cosT])).astype(BF)
    ssk2 = np.ascontiguousarray(np.vstack([-sinT, sinT])).astype(BF)

    m2 = mask[0, 0]  # [S(q), S(k)] additive
    # widened tip mask [128(k), 256(q)]: first 128 q-cols all-invalid (zeros),
    # last 128 = diagonal tri block (identical for every causal diag block)
    tri_d = np.exp(m2[7 * 128:8 * 128, 7 * 128:8 * 128]).T.astype(np.float32)
    triw = np.concatenate([np.zeros((128, 128), np.float32), tri_d],
                          axis=1).astype(BF)
    ones128 = np.ones((128, 128), np.float32)

    in_maps = []
    for c in range(N_CORES):
        b, g = c // 2, c % 2
        gp = 1 - g
        hs = slice(HPC * g, HPC * g + HPC)
        wv_pre = np.ascontiguousarray(
            wvT[:, g * 1024:(g + 1) * 1024]
            .reshape(KC, 128, NP, 256).transpose(2, 1, 0, 3))
        woh = woT[:, g * 1024:(g + 1) * 1024].reshape(KC, 128, EHALF)
        wol_pre = np.ascontiguousarray(np.stack(
            [np.stack([woh[8 * g + 2 * p], woh[8 * g + 2 * p + 1]], axis=1)
             for p in range(NP)], axis=0))
        wor_pre = np.ascontiguousarray(np.stack(
            [np.stack([woh[8 * gp + 2 * p], woh[8 * gp + 2 * p + 1]], axis=1)
             for p in range(NP)], axis=0))
        bq_p = np.ascontiguousarray(bq_all[hs].T)[:, :, None]  # [128, HPC, 1]
        bk_p = np.ascontiguousarray(bk_all[hs].T)[:, :, None]
        bv_p = np.ascontiguousarray(np.broadcast_to(
            bv[g * 1024:(g + 1) * 1024].reshape(NP, 256)[None],
            (128, NP, 256))).astype(BF)
        in_maps.append({
            "x_pre": _pack_x(x[b]),
            "wq_pre": np.ascontiguousarray(wq_all[hs]),
            "wk_pre": np.ascontiguousarray(wk_all[hs]),
            "wv_pre": wv_pre,
            "wol_pre": wol_pre,
            "wor_pre": wor_pre,
            "bq_p": bq_p, "bk_p": bk_p, "bv_p": bv_p,
            "csk2": csk2, "ssk2": ssk2,
            "triw": triw, "ones128": ones128,
            "sel_lo": np.array([[1 if g == 1 else 0]], np.uint32),
            "sel_hi": np.array([[1 if g == 0 else 0]], np.uint32),
        })
    return in_maps


def kernel(**inputs):
    from concourse.bass_utils import run_bass_kernel_spmd

    trace = bool(inputs.pop("_trace", False))
    bo = np.asarray(inputs["bo"], np.float32)
    in_maps = _prep_inputs(inputs)

    nc = _get_nc()
    kwargs = {}
    if trace:
        kwargs = {"trace": True, "trace_cores": list(range(N_CORES))}
    res = run_bass_kernel_spmd(nc, in_maps, core_ids=list(range(N_CORES)), **kwargs)
    _cache["last_result"] = res

    out = np.empty((B, S, DIM), np.float32)
    for c in range(N_CORES):
        b, g = c // 2, c % 2
        out[b, :, g * 1024:(g + 1) * 1024] = (
            res.results[c]["y"] + bo[None, g * 1024:(g + 1) * 1024])
    return out


# revision 16
# speedup vs baseline: 1.0696x; 1.0696x over previous
"""Multi-head causal attention (LLaMA RoPE), head-parallel sharding on 8 trn2 cores.

Core c = (batch b=c//2, head-group g=c%2). Each core computes Q/K/V projections
and attention for its 8 heads over the FULL 1024 rows of its batch (zero
duplication). All matmuls run fp32r: on TRN2 fp32r streams 1 row/cycle at
free-size >= 256 while bf16 takes 2 cycles/row, so fp32r is the fast path
everywhere including the O-projection (the baseline's bf16 o_pass paid 2x).

O-projection is split local/remote: each pair's attention output (ot) stays
SBUF-resident and its y contribution accumulates with no collective
dependency; only the peer head-group's ot crosses the wire (pair AllGather
via DRAM), then a conditional DMA (cond = host-supplied group flag, keeping
the SPMD program symmetric) pulls just the peer rows back to SBUF for the
remote half. Two HWDGE queues: sync carries input streams (x, weights,
tables), scalar/Act carries exchange+output streams (ot, sot, y), so
collectives never queue behind weight prefetches.

Scores are computed transposed ST[k,q]; softmax skips max-subtraction;
denominator via ones-matmul accumulated alongside PV; causal mask applied
post-exp as a binary multiply on diagonal 128x128 tiles (widened to 256 on
the narrow tips so fp32r stays on its fast path). Attention issue order is
software-pipelined (2-deep ST lookahead) so the PE rides the Act exp chain
instead of waiting on it. x and pair-0 weights stream in fine-grained
interleaved chunks so the PE starts ~2us in.
"""

import math
import sys

import numpy as np

sys.path.insert(0, "/opt/trn_rl_repo")

B, S, DIM, H = 4, 1024, 2048, 16
HD = DIM // H  # 128
KC = DIM // 128  # 16 contraction chunks
HPC = H // 2  # 8 heads per core
NP = HPC // 2  # 4 head-pairs per core
EHALF = DIM // 2  # 1024 output cols per core
SCALE = 1.0 / math.sqrt(HD)
N_CORES = 8
PAIRS = [[0, 1], [2, 3], [4, 5], [6, 7]]

_cache = {}


def _build_nc():
    import concourse.mybir as mybir
    import concourse.tile as tile
    from concourse import bacc

    F32R = mybir.dt.float32r
    F32 = mybir.dt.float32
    BF16 = mybir.dt.bfloat16
    U32 = mybir.dt.uint32

    nc = bacc.Bacc("TRN2", target_bir_lowering=False, debug=False,
                   num_devices=N_CORES)

    x_in = nc.dram_tensor("x_pre", [2, 128, KC, 512], F32R, kind="ExternalInput")
    wq_in = nc.dram_tensor("wq_pre", [HPC, 128, KC, 128], F32R, kind="ExternalInput")
    wk_in = nc.dram_tensor("wk_pre", [HPC, 128, KC, 128], F32R, kind="ExternalInput")
    wv_in = nc.dram_tensor("wv_pre", [NP, 128, KC, 256], F32R, kind="ExternalInput")
    wol_in = nc.dram_tensor("wol_pre", [NP, 128, 2, EHALF], F32R, kind="ExternalInput")
    wor_in = nc.dram_tensor("wor_pre", [NP, 128, 2, EHALF], F32R, kind="ExternalInput")
    bq_in = nc.dram_tensor("bq_p", [128, HPC, 1], F32, kind="ExternalInput")
    bk_in = nc.dram_tensor("bk_p", [128, HPC, 1], F32, kind="ExternalInput")
    bv_in = nc.dram_tensor("bv_p", [128, NP, 256], BF16, kind="ExternalInput")
    csk_in = nc.dram_tensor("csk2", [128, S], BF16, kind="ExternalInput")
    ssk_in = nc.dram_tensor("ssk2", [128, S], BF16, kind="ExternalInput")
    triw_in = nc.dram_tensor("triw", [128, 256], BF16, kind="ExternalInput")
    ones_in = nc.dram_tensor("ones128", [128, 128], F32R, kind="ExternalInput")
    sel_lo_in = nc.dram_tensor("sel_lo", [1, 1], U32, kind="ExternalInput")
    sel_hi_in = nc.dram_tensor("sel_hi", [1, 1], U32, kind="ExternalInput")
    y_out = nc.dram_tensor("y", [S, EHALF], F32, kind="ExternalOutput")

    with tile.TileContext(nc) as tc:
        with (
            tc.tile_pool(name="consts", bufs=1) as consts,
            tc.tile_pool(name="xpool", bufs=1) as xpool,
            tc.tile_pool(name="wpool", bufs=1) as wpool,
            tc.tile_pool(name="qkv", bufs=1) as qkv,
            tc.tile_pool(name="rope", bufs=2) as rope,
            tc.tile_pool(name="ptp", bufs=2) as ptp,
            tc.tile_pool(name="otsp", bufs=1) as otsp,
            tc.tile_pool(name="sotp", bufs=1) as sotp,
            tc.tile_pool(name="yap", bufs=1) as yap,
            tc.tile_pool(name="ysp", bufs=2) as ysp,
            tc.tile_pool(name="dram", bufs=1, space="DRAM") as dram,
            tc.tile_pool(name="ps_qk", bufs=2, space="PSUM") as ps_qk,
            tc.tile_pool(name="ps_st", bufs=2, space="PSUM") as ps_st,
            tc.tile_pool(name="ps_acc", bufs=4, space="PSUM") as ps_acc,
        ):
            # group-select flags for the conditional peer-block DMA
            r_lo = nc.scalar.alloc_register("sel_lo_r")
            nc.scalar.reg_load(r_lo, sel_lo_in[0:1, 0:1])
            sel_lo = nc.scalar.snap(r_lo, donate=True, min_val=0, max_val=1)
            r_hi = nc.scalar.alloc_register("sel_hi_r")
            nc.scalar.reg_load(r_hi, sel_hi_in[0:1, 0:1])
            sel_hi = nc.scalar.snap(r_hi, donate=True, min_val=0, max_val=1)
            g_lo = nc.gpsimd.alloc_register("sel_lo_g")
            nc.gpsimd.reg_load(g_lo, sel_lo_in[0:1, 0:1])
            sel_lo_g = nc.gpsimd.snap(g_lo, donate=True, min_val=0, max_val=1)
            g_hi = nc.gpsimd.alloc_register("sel_hi_g")
            nc.gpsimd.reg_load(g_hi, sel_hi_in[0:1, 0:1])
            sel_hi_g = nc.gpsimd.snap(g_hi, donate=True, min_val=0, max_val=1)

            # ---- startup: interleaved x chunks + pair-0 weights on sync q ----
            bq_sb = consts.tile([128, HPC, 1], F32, tag="bq")
            bk_sb = consts.tile([128, HPC, 1], F32, tag="bk")
            nc.sync.dma_start(bq_sb[:], bq_in[:])
            nc.sync.dma_start(bk_sb[:], bk_in[:])

            x_sb = xpool.tile([128, KC, S], F32R, tag="x", name="x_sb")

            def x_dma(h, kcs):
                for kc in kcs:
                    nc.sync.dma_start(x_sb[:, kc, h * 512:h * 512 + 512],
                                      x_in[h, :, kc, :])

            wq_sb = {}
            wk_sb = {}

            def load_qk(p, lh, split_x=None):
                wq_sb[(p, lh)] = wpool.tile([128, KC, 128], F32R,
                                            tag=f"wq{lh}", name="wq_sb")
                wk_sb[(p, lh)] = wpool.tile([128, KC, 128], F32R,
                                            tag=f"wk{lh}", name="wk_sb")
                nc.sync.dma_start(wq_sb[(p, lh)][:], wq_in[2 * p + lh])
                if split_x is not None:
                    split_x(0, range(8 * lh, 8 * lh + 8))
                nc.sync.dma_start(wk_sb[(p, lh)][:], wk_in[2 * p + lh])

            wv_sb = {}
            wol_sb = {}
            wor_sb = {}

            def load_v(p):
                wv_sb[p] = wpool.tile([128, KC, 256], F32R, tag="wv",
                                      name="wv_sb")
                nc.sync.dma_start(wv_sb[p][:], wv_in[p])

            def load_wo(p):
                wol_sb[p] = wpool.tile([128, 2, EHALF], F32R, tag="wol",
                                       name="wol_sb")
                wor_sb[p] = wpool.tile([128, 2, EHALF], F32R, tag="wor",
                                       name="wor_sb")
                nc.sync.dma_start(wol_sb[p][:], wol_in[p])
                nc.sync.dma_start(wor_sb[p][:], wor_in[p])

            load_qk(0, 0, split_x=x_dma)
            load_qk(0, 1, split_x=x_dma)
            csk_sb = consts.tile([128, S], BF16, tag="cs", name="csk_sb")
            ssk_sb = consts.tile([128, S], BF16, tag="ss", name="ssk_sb")
            nc.sync.dma_start(csk_sb[:], csk_in[:])
            nc.sync.dma_start(ssk_sb[:], ssk_in[:])
            load_v(0)
            x_dma(1, range(0, 8))
            bv_sb = consts.tile([128, NP, 256], BF16, tag="bv")
            nc.sync.dma_start(bv_sb[:], bv_in[:])
            triw_sb = consts.tile([128, 256], BF16, tag="triw")
            nc.sync.dma_start(triw_sb[:], triw_in[:])
            ones_sb = consts.tile([128, 128], F32R, tag="ones")
            nc.sync.dma_start(ones_sb[:], ones_in[:])
            x_dma(1, range(8, 16))
            load_wo(0)

            # ---- DRAM tiles for the ot exchange ----
            ag_ins = []
            ag_outs = []
            for p in range(NP - 1):
                ag_ins.append(dram.tile([256, S], F32R, name=f"agi{p}"))
                ag_outs.append(dram.tile([4, 128, S], F32R, name=f"ago{p}"))
            ag3_ins = [dram.tile([256, 512], F32R, name=f"agi3q{qc}")
                       for qc in range(2)]
            ag3_outs = [dram.tile([4, 128, 512], F32R, name=f"ago3q{qc}")
                        for qc in range(2)]

            y_acc = yap.tile([128, 8, 2, 512], BF16, name="y_acc")
            ots = {}  # (p, lh, qc) -> SBUF tile [128, 512] f32r
            sotr = {}  # p -> [128, 2, S] (p<3) / (3, qc) -> [128, 2, 512]

            def rope_chunk(pm, b_ap, cs_ap, ss_ap, dst_ap):
                """dst = rope(pm + bias); all [128, 512]."""
                tmp = rope.tile([128, 512], BF16, tag="tmp", name="tmp")
                nc.scalar.activation(
                    tmp[:], pm, mybir.ActivationFunctionType.Identity, bias=b_ap
                )
                tsw = rope.tile([128, 512], BF16, tag="tsw", name="tsw")
                nc.scalar.dma_start(tsw[0:64, :], tmp[64:128, :])
                nc.scalar.dma_start(tsw[64:128, :], tmp[0:64, :])
                nc.vector.tensor_mul(tmp[:], tmp[:], cs_ap)
                nc.vector.tensor_mul(tsw[:], tsw[:], ss_ap)
                nc.vector.tensor_add(dst_ap, tmp[:], tsw[:])

            def qk_proj(p):
                q_sb = [qkv.tile([128, S], F32R, tag=f"q{lh}", name="q_sb")
                        for lh in range(2)]
                k_sb = [qkv.tile([128, S], F32R, tag=f"k{lh}", name="k_sb")
                        for lh in range(2)]
                for lh in range(2):
                    h = 2 * p + lh
                    # c-major so the startup pair consumes x in arrival order
                    for c in range(2):
                        ccols = slice(c * 512, c * 512 + 512)
                        for dst, w_sb, b_sb in (
                            (q_sb[lh], wq_sb[(p, lh)], bq_sb),
                            (k_sb[lh], wk_sb[(p, lh)], bk_sb),
                        ):
                            pm = ps_qk.tile([128, 512], F32, tag="qk", name="pm")
                            for kc in range(KC):
                                nc.tensor.matmul(pm[:], w_sb[:, kc, :],
                                                 x_sb[:, kc, ccols],
                                                 start=(kc == 0),
                                                 stop=(kc == KC - 1))
                            rope_chunk(pm[:], b_sb[:, h, :], csk_sb[:, ccols],
                                       ssk_sb[:, ccols], dst[:, ccols])
                return q_sb, k_sb

            def v_proj(p):
                v_sb = qkv.tile([128, 8, 256], F32R, tag="v", name="v_sb")
                for st in range(8):
                    scols = slice(st * 128, st * 128 + 128)
                    vp = ps_qk.tile([128, 512], F32, tag="qk", name="vp")
                    for kc in range(KC):
                        nc.tensor.matmul(vp[:, 0:256], x_sb[:, kc, scols],
                                         wv_sb[p][:, kc, :],
                                         start=(kc == 0), stop=(kc == KC - 1))
                    nc.vector.tensor_add(v_sb[:, st, :], vp[:, 0:256],
                                         bv_sb[:, p, :])
                return v_sb

            def attn_block(p, lh, qc, q_sb, k_sb, v_sb):
                """One (lh, qc) attention block, software-pipelined issue.

                Narrow 128-wide tips are widened to 256 (fp32r fast path);
                the extra columns are fully-masked garbage zeroed by the
                widened mask, harmless to the l/o accumulations.
                """
                infos = []
                kts = range(4) if qc == 0 else range(8)
                for kt in kts:
                    if qc == 0:
                        qv = slice(kt * 128, 512)
                        dg = slice(kt * 128, kt * 128 + 128)
                    else:
                        qv = slice((kt - 4) * 128, 512) if kt >= 4 \
                            else slice(0, 512)
                        dg = slice((kt - 4) * 128, (kt - 4) * 128 + 128) \
                            if kt >= 4 else None
                    wide = qv.stop - qv.start == 128
                    if wide:
                        qv = slice(qv.start - 128, 512)
                        dg = qv
                    infos.append((kt, qv, dg, wide))
                n = len(infos)
                vcols = slice(lh * 128, lh * 128 + 128)
                l_ps = ps_acc.tile([128, 512], F32, tag="acc", name="l_ps")
                o_ps = ps_acc.tile([128, 512], F32, tag="acc", name="o_ps")
                PIPE = 2
                pts = {}
                for i in range(n + PIPE):
                    if i < n:
                        kt, qv, dg, wide = infos[i]
                        qg = slice(qc * 512 + qv.start, qc * 512 + 512)
                        st_ps = ps_st.tile([128, 512], F32, tag="st",
                                           name="st_ps")
                        nc.tensor.matmul(st_ps[:, qv],
                                         k_sb[lh][:, kt * 128:kt * 128 + 128],
                                         q_sb[lh][:, qg], start=True, stop=True)
                        pt = ptp.tile([128, 512], F32R, tag="pt", name="pt")
                        nc.scalar.activation(pt[:, qv], st_ps[:, qv],
                                             mybir.ActivationFunctionType.Exp,
                                             scale=SCALE)
                        if dg is not None:
                            msk = triw_sb[:] if wide else triw_sb[:, 128:256]
                            nc.vector.tensor_mul(pt[:, dg], pt[:, dg], msk)
                        pts[i] = (pt, qv)
                    j = i - PIPE
                    if 0 <= j < n:
                        pt, qv = pts.pop(j)
                        ktj = infos[j][0]
                        first, last = (j == 0), (j == n - 1)
                        nc.tensor.matmul(l_ps[:, qv], ones_sb[:], pt[:, qv],
                                         start=first, stop=last)
                        nc.tensor.matmul(o_ps[:, qv], v_sb[:, ktj, vcols],
                                         pt[:, qv], start=first, stop=last)
                rl = ysp.tile([128, 512], F32, tag="rl", bufs=1, name="rl")
                nc.vector.reciprocal_approx_fast(rl[:], l_ps[:])
                ot = otsp.tile([128, 512], F32R, tag=f"ot{lh}{qc}", name="ot")
                nc.vector.tensor_mul(ot[:], o_ps[:], rl[:])
                ots[(p, lh, qc)] = ot
                # exchange input on the Act queue
                if p < 3:
                    nc.scalar.dma_start(
                        ag_ins[p][lh * 128:lh * 128 + 128,
                                  qc * 512:qc * 512 + 512], ot[:])
                else:
                    nc.scalar.dma_start(
                        ag3_ins[qc][lh * 128:lh * 128 + 128, :], ot[:])

            def fetch_remote(p, qc=None, eng=None):
                """Conditional DMA of the peer head-group's ot rows to SBUF."""
                if p < 3:
                    w, tag = S, "sotr"
                    src = ag_outs[p].rearrange("a r c -> r a c")
                else:
                    w, tag = 512, "sotr3"
                    src = ag3_outs[qc].rearrange("a r c -> r a c")
                t = sotp.tile([128, 2, w], F32R, tag=tag, name="sotr_t")
                if eng is None:
                    nc.scalar.dma_start(t[:], src[:, 0:2, :], cond=sel_lo)
                    nc.scalar.dma_start(t[:], src[:, 2:4, :], cond=sel_hi)
                else:
                    eng.dma_start(t[:], src[:, 0:2, :], cond=sel_lo_g)
                    eng.dma_start(t[:], src[:, 2:4, :], cond=sel_hi_g)
                sotr[p if p < 3 else (3, qc)] = t

            def o_comb(p):
                """Accumulate local pair-p + remote pair-(p-1) into y_acc."""
                for qt in range(8):
                    qc, col = qt // 4, (qt % 4) * 128
                    for eb in range(2):
                        ecols = slice(eb * 512, eb * 512 + 512)
                        pool = ps_qk if (qt * 2 + eb) % 2 == 0 else ps_st
                        tag = "qk" if (qt * 2 + eb) % 2 == 0 else "st"
                        y_ps = pool.tile([128, 512], F32, tag=tag,
                                         name="y_ps")
                        nmm = 2 if p == 0 else 4
                        mm = 0
                        for a in range(2):
                            mm += 1
                            nc.tensor.matmul(
                                y_ps[:], ots[(p, a, qc)][:, col:col + 128],
                                wol_sb[p][:, a, ecols],
                                start=(mm == 1), stop=(mm == nmm))
                        if p > 0:
                            for a in range(2):
                                mm += 1
                                nc.tensor.matmul(
                                    y_ps[:],
                                    sotr[p - 1][:, a, qt * 128:qt * 128 + 128],
                                    wor_sb[p - 1][:, a, ecols],
                                    start=False, stop=(mm == nmm))
                        ya = y_acc[:, qt, eb, :]
                        if p == 0:
                            nc.vector.tensor_copy(ya, y_ps[:])
                        else:
                            nc.vector.tensor_add(ya, ya, y_ps[:])

            def o_final(qts, sot3):
                """Remote pair-3 chunks + final add and store.

                y_sb cycles through 3 borrowed buffers (pt x2 + rl) so the
                y-store DMA completion latency pipelines instead of
                serializing; y_ps alternates the qk/st PSUM pools for
                rotation slack."""
                for qt in qts:
                    col = (qt % 4) * 128
                    qcols = slice(qt * 128, qt * 128 + 128)
                    for eb in range(2):
                        ecols = slice(eb * 512, eb * 512 + 512)
                        pool = ps_qk if (qt * 2 + eb) % 2 == 0 else ps_st
                        tag = "qk" if (qt * 2 + eb) % 2 == 0 else "st"
                        y_ps = pool.tile([128, 512], F32, tag=tag,
                                         name="y_ps")
                        for a in range(2):
                            nc.tensor.matmul(
                                y_ps[:], sot3[:, a, col:col + 128],
                                wor_sb[3][:, a, ecols],
                                start=(a == 0), stop=(a == 1))
                        r = (qt * 2 + eb) % 3
                        if r < 2:
                            y_sb = ptp.tile([128, 512], F32, tag="pt",
                                            name="y_sb")
                        else:
                            y_sb = ysp.tile([128, 512], F32, tag="rl",
                                            bufs=1, name="y_sb")
                        nc.vector.tensor_add(y_sb[:], y_acc[:, qt, eb, :],
                                             y_ps[:])
                        nc.scalar.dma_start(y_out[qcols, ecols], y_sb[:])

            # ================= main pair loop =================
            q_sb = k_sb = v_sb = None
            for p in range(NP):
                if p == 0:
                    q_sb, k_sb = qk_proj(0)
                    v_sb = v_proj(0)
                if p + 1 < NP:
                    load_qk(p + 1, 0)
                    load_qk(p + 1, 1)
                    load_v(p + 1)
                if p > 0:
                    load_wo(p)

                # ---- attention ----
                lq = [(lh, qc) for lh in range(2) for qc in range(2)] \
                    if p < 3 else [(lh, qc) for qc in range(2) for lh in range(2)]
                for bi, (lh, qc) in enumerate(lq):
                    attn_block(p, lh, qc, q_sb, k_sb, v_sb)
                    # fetch the previous pair's peer rows mid-attention: its
                    # AllGather finished long ago, so the act queue never
                    # blocks, and the data lands well before o_comb(p)
                    if bi == 0 and p > 0:
                        fetch_remote(p - 1)
                    if p == 3 and lh == 1:
                        nc.gpsimd.collective_compute(
                            "AllGather", mybir.AluOpType.bypass,
                            replica_groups=PAIRS,
                            ins=[ag3_ins[qc].opt()],
                            outs=[ag3_outs[qc].opt()],
                        )
                if p < 3:
                    nc.gpsimd.collective_compute(
                        "AllGather", mybir.AluOpType.bypass, replica_groups=PAIRS,
                        ins=[ag_ins[p].opt()], outs=[ag_outs[p].opt()],
                    )

                # ---- next proj, then o accumulation, then V ----
                if p + 1 < NP:
                    nq, nk = qk_proj(p + 1)
                    o_comb(p)
                    nv = v_proj(p + 1)
                    q_sb, k_sb, v_sb = nq, nk, nv
                else:
                    o_comb(3)
                    # fetches sit after o_comb so the act queue never blocks
                    # on AG3 waits while attention exps are still pending
                    fetch_remote(3, 0)
                    o_final(range(0, 4), sotr[(3, 0)])
                    fetch_remote(3, 1)
                    o_final(range(4, 8), sotr[(3, 1)])
    nc.compile()
    return nc


def _get_nc():
    if "nc" not in _cache:
        _cache["nc"] = _build_nc()
    return _cache["nc"]


def _head_perm():
    p = []
    for h in range(H):
        base = h * HD
        p += [base + 2 * j for j in range(HD // 2)]
        p += [base + 2 * j + 1 for j in range(HD // 2)]
    return np.array(p)


def _pack_thin(wT):
    # [2048(k), 2048(d)] -> [H, 128(p), KC, 128(d)]
    return np.ascontiguousarray(
        wT.reshape(KC, 128, H, 128).transpose(2, 1, 0, 3)
    )


def _pack_x(xb):
    # [rows 1024, 2048] -> [2(col-half), 128(p), KC, 512] (contiguous halves)
    xt = xb.T.reshape(KC, 128, 2, 512)
    return np.ascontiguousarray(xt.transpose(2, 1, 0, 3))


def _prep_inputs(inputs):
    import ml_dtypes

    BF = ml_dtypes.bfloat16
    x = np.asarray(inputs["x"], np.float32)
    freqs_cos = np.asarray(inputs["freqs_cos"], np.float32)
    freqs_sin = np.asarray(inputs["freqs_sin"], np.float32)
    mask = np.asarray(inputs["mask"], np.float32)
    wq = np.asarray(inputs["wq"], np.float32)
    bq = np.asarray(inputs["bq"], np.float32)
    wk = np.asarray(inputs["wk"], np.float32)
    bk = np.asarray(inputs["bk"], np.float32)
    wv = np.asarray(inputs["wv"], np.float32)
    bv = np.asarray(inputs["bv"], np.float32)
    wo = np.asarray(inputs["wo"], np.float32)
    start_pos = int(np.asarray(inputs.get("start_pos", 0)))

    perm = _head_perm()
    wq_all = _pack_thin(np.ascontiguousarray(wq[perm].T))  # [H,128,KC,128]
    wk_all = _pack_thin(np.ascontiguousarray(wk[perm].T))
    bq_all = bq[perm].reshape(H, 128)  # [H, 128]
    bk_all = bk[perm].reshape(H, 128)

    wvT = np.ascontiguousarray(wv.T)  # [in 2048, out 2048]
    woT = np.ascontiguousarray(wo.T)  # [d 2048, e 2048]

    cosT = freqs_cos[start_pos:start_pos + S].T.astype(np.float32)  # [64, S]
    sinT = freqs_sin[start_pos:start_pos + S].T.astype(np.float32)
    csk2 = np.ascontiguousarray(np.vstack([cosT, cosT])).astype(BF)
    ssk2 = np.ascontiguousarray(np.vstack([-sinT, sinT])).astype(BF)

    m2 = mask[0, 0]  # [S(q), S(k)] additive
    # widened tip mask [128(k), 256(q)]: first 128 q-cols all-invalid (zeros),
    # last 128 = diagonal tri block (identical for every causal diag block)
    tri_d = np.exp(m2[7 * 128:8 * 128, 7 * 128:8 * 128]).T.astype(np.float32)
    triw = np.concatenate([np.zeros((128, 128), np.float32), tri_d],
                          axis=1).astype(BF)
    ones128 = np.ones((128, 128), np.float32)

    in_maps = []
    for c in range(N_CORES):
        b, g = c // 2, c % 2
        gp = 1 - g
        hs = slice(HPC * g, HPC * g + HPC)
        wv_pre = np.ascontiguousarray(
            wvT[:, g * 1024:(g + 1) * 1024]
            .reshape(KC, 128, NP, 256).transpose(2, 1, 0, 3))
        woh = woT[:, g * 1024:(g + 1) * 1024].reshape(KC, 128, EHALF)
        wol_pre = np.ascontiguousarray(np.stack(
            [np.stack([woh[8 * g + 2 * p], woh[8 * g + 2 * p + 1]], axis=1)
             for p in range(NP)], axis=0))
        wor_pre = np.ascontiguousarray(np.stack(
            [np.stack([woh[8 * gp + 2 * p], woh[8 * gp + 2 * p + 1]], axis=1)
             for p in range(NP)], axis=0))
        bq_p = np.ascontiguousarray(bq_all[hs].T)[:, :, None]  # [128, HPC, 1]
        bk_p = np.ascontiguousarray(bk_all[hs].T)[:, :, None]
        bv_p = np.ascontiguousarray(np.broadcast_to(
            bv[g * 1024:(g + 1) * 1024].reshape(NP, 256)[None],
            (128, NP, 256))).astype(BF)
        in_maps.append({
            "x_pre": _pack_x(x[b]),
            "wq_pre": np.ascontiguousarray(wq_all[hs]),
            "wk_pre": np.ascontiguousarray(wk_all[hs]),
            "wv_pre": wv_pre,
            "wol_pre": wol_pre,
            "wor_pre": wor_pre,
            "bq_p": bq_p, "bk_p": bk_p, "bv_p": bv_p,
            "csk2": csk2, "ssk2": ssk2,
            "triw": triw, "ones128": ones128,
            "sel_lo": np.array([[1 if g == 1 else 0]], np.uint32),
            "sel_hi": np.array([[1 if g == 0 else 0]], np.uint32),
        })
    return in_maps


def kernel(**inputs):
    from concourse.bass_utils import run_bass_kernel_spmd

    trace = bool(inputs.pop("_trace", False))
    bo = np.asarray(inputs["bo"], np.float32)
    in_maps = _prep_inputs(inputs)

    nc = _get_nc()
    kwargs = {}
    if trace:
        kwargs = {"trace": True, "trace_cores": list(range(N_CORES))}
    res = run_bass_kernel_spmd(nc, in_maps, core_ids=list(range(N_CORES)), **kwargs)
    _cache["last_result"] = res

    out = np.empty((B, S, DIM), np.float32)
    for c in range(N_CORES):
        b, g = c // 2, c % 2
        out[b, :, g * 1024:(g + 1) * 1024] = (
            res.results[c]["y"] + bo[None, g * 1024:(g + 1) * 1024])
    return out


# revision 17
# speedup vs baseline: 1.0737x; 1.0038x over previous
"""Multi-head causal attention (LLaMA RoPE), head-parallel sharding on 8 trn2 cores.

Core c = (batch b=c//2, head-group g=c%2). Each core computes Q/K/V projections
and attention for its 8 heads over the FULL 1024 rows of its batch (zero
duplication). All matmuls run fp32r: on TRN2 fp32r streams 1 row/cycle at
free-size >= 256 while bf16 takes 2 cycles/row, so fp32r is the fast path
everywhere including the O-projection (the baseline's bf16 o_pass paid 2x).

O-projection is split local/remote: each pair's attention output (ot) stays
SBUF-resident and its y contribution accumulates with no collective
dependency; only the peer head-group's ot crosses the wire (pair AllGather
via DRAM), then a conditional DMA (cond = host-supplied group flag, keeping
the SPMD program symmetric) pulls just the peer rows back to SBUF for the
remote half. Two HWDGE queues: sync carries input streams (x, weights,
tables), scalar/Act carries exchange+output streams (ot, sot, y), so
collectives never queue behind weight prefetches.

Scores are computed transposed ST[k,q]; softmax skips max-subtraction;
denominator via ones-matmul accumulated alongside PV; causal mask applied
post-exp as a binary multiply on diagonal 128x128 tiles (widened to 256 on
the narrow tips so fp32r stays on its fast path). Attention issue order is
software-pipelined (2-deep ST lookahead) so the PE rides the Act exp chain
instead of waiting on it. x and pair-0 weights stream in fine-grained
interleaved chunks so the PE starts ~2us in.
"""

import math
import sys

import numpy as np

sys.path.insert(0, "/opt/trn_rl_repo")

B, S, DIM, H = 4, 1024, 2048, 16
HD = DIM // H  # 128
KC = DIM // 128  # 16 contraction chunks
HPC = H // 2  # 8 heads per core
NP = HPC // 2  # 4 head-pairs per core
EHALF = DIM // 2  # 1024 output cols per core
SCALE = 1.0 / math.sqrt(HD)
N_CORES = 8
PAIRS = [[0, 1], [2, 3], [4, 5], [6, 7]]

_cache = {}


def _build_nc():
    import concourse.mybir as mybir
    import concourse.tile as tile
    from concourse import bacc

    F32R = mybir.dt.float32r
    F32 = mybir.dt.float32
    BF16 = mybir.dt.bfloat16
    U32 = mybir.dt.uint32

    nc = bacc.Bacc("TRN2", target_bir_lowering=False, debug=False,
                   num_devices=N_CORES)

    x_in = nc.dram_tensor("x_pre", [2, 128, KC, 512], F32R, kind="ExternalInput")
    wq_in = nc.dram_tensor("wq_pre", [HPC, 128, KC, 128], F32R, kind="ExternalInput")
    wk_in = nc.dram_tensor("wk_pre", [HPC, 128, KC, 128], F32R, kind="ExternalInput")
    wv_in = nc.dram_tensor("wv_pre", [NP, 128, KC, 256], F32R, kind="ExternalInput")
    wol_in = nc.dram_tensor("wol_pre", [NP, 128, 2, EHALF], F32R, kind="ExternalInput")
    wor_in = nc.dram_tensor("wor_pre", [NP, 128, 2, EHALF], F32R, kind="ExternalInput")
    bq_in = nc.dram_tensor("bq_p", [128, HPC, 1], F32, kind="ExternalInput")
    bk_in = nc.dram_tensor("bk_p", [128, HPC, 1], F32, kind="ExternalInput")
    bv_in = nc.dram_tensor("bv_p", [128, NP, 256], BF16, kind="ExternalInput")
    csk_in = nc.dram_tensor("csk2", [128, S], BF16, kind="ExternalInput")
    ssk_in = nc.dram_tensor("ssk2", [128, S], BF16, kind="ExternalInput")
    triw_in = nc.dram_tensor("triw", [128, 256], BF16, kind="ExternalInput")
    ones_in = nc.dram_tensor("ones128", [128, 128], F32R, kind="ExternalInput")
    sel_lo_in = nc.dram_tensor("sel_lo", [1, 1], U32, kind="ExternalInput")
    sel_hi_in = nc.dram_tensor("sel_hi", [1, 1], U32, kind="ExternalInput")
    y_out = nc.dram_tensor("y", [S, EHALF], F32, kind="ExternalOutput")

    with tile.TileContext(nc) as tc:
        with (
            tc.tile_pool(name="consts", bufs=1) as consts,
            tc.tile_pool(name="xpool", bufs=1) as xpool,
            tc.tile_pool(name="wpool", bufs=1) as wpool,
            tc.tile_pool(name="qkv", bufs=1) as qkv,
            tc.tile_pool(name="rope", bufs=2) as rope,
            tc.tile_pool(name="ptp", bufs=2) as ptp,
            tc.tile_pool(name="otsp", bufs=1) as otsp,
            tc.tile_pool(name="sotp", bufs=1) as sotp,
            tc.tile_pool(name="yap", bufs=1) as yap,
            tc.tile_pool(name="ysp", bufs=2) as ysp,
            tc.tile_pool(name="dram", bufs=1, space="DRAM") as dram,
            tc.tile_pool(name="ps_qk", bufs=2, space="PSUM") as ps_qk,
            tc.tile_pool(name="ps_st", bufs=2, space="PSUM") as ps_st,
            tc.tile_pool(name="ps_acc", bufs=4, space="PSUM") as ps_acc,
        ):
            # group-select flags for the conditional peer-block DMA
            r_lo = nc.scalar.alloc_register("sel_lo_r")
            nc.scalar.reg_load(r_lo, sel_lo_in[0:1, 0:1])
            sel_lo = nc.scalar.snap(r_lo, donate=True, min_val=0, max_val=1)
            r_hi = nc.scalar.alloc_register("sel_hi_r")
            nc.scalar.reg_load(r_hi, sel_hi_in[0:1, 0:1])
            sel_hi = nc.scalar.snap(r_hi, donate=True, min_val=0, max_val=1)
            g_lo = nc.gpsimd.alloc_register("sel_lo_g")
            nc.gpsimd.reg_load(g_lo, sel_lo_in[0:1, 0:1])
            sel_lo_g = nc.gpsimd.snap(g_lo, donate=True, min_val=0, max_val=1)
            g_hi = nc.gpsimd.alloc_register("sel_hi_g")
            nc.gpsimd.reg_load(g_hi, sel_hi_in[0:1, 0:1])
            sel_hi_g = nc.gpsimd.snap(g_hi, donate=True, min_val=0, max_val=1)

            # ---- startup: interleaved x chunks + pair-0 weights on sync q ----
            bq_sb = consts.tile([128, HPC, 1], F32, tag="bq")
            bk_sb = consts.tile([128, HPC, 1], F32, tag="bk")
            nc.sync.dma_start(bq_sb[:], bq_in[:])
            nc.sync.dma_start(bk_sb[:], bk_in[:])

            x_sb = xpool.tile([128, KC, S], F32R, tag="x", name="x_sb")

            def x_dma(h, kc0, kc1):
                nc.sync.dma_start(x_sb[:, kc0:kc1, h * 512:h * 512 + 512],
                                  x_in[h, :, kc0:kc1, :])

            wq_sb = {}
            wk_sb = {}

            def load_qk(p, lh, split_x=None):
                wq_sb[(p, lh)] = wpool.tile([128, KC, 128], F32R,
                                            tag=f"wq{lh}", name="wq_sb")
                wk_sb[(p, lh)] = wpool.tile([128, KC, 128], F32R,
                                            tag=f"wk{lh}", name="wk_sb")
                nc.sync.dma_start(wq_sb[(p, lh)][:], wq_in[2 * p + lh])
                if split_x is not None:
                    split_x(0, 8 * lh, 8 * lh + 4)
                nc.sync.dma_start(wk_sb[(p, lh)][:], wk_in[2 * p + lh])
                if split_x is not None:
                    split_x(0, 8 * lh + 4, 8 * lh + 8)

            wv_sb = {}
            wol_sb = {}
            wor_sb = {}

            def load_v(p):
                wv_sb[p] = wpool.tile([128, KC, 256], F32R, tag="wv",
                                      name="wv_sb")
                nc.sync.dma_start(wv_sb[p][:], wv_in[p])

            def load_wo(p, eng=None):
                eng = eng or nc.sync
                wol_sb[p] = wpool.tile([128, 2, EHALF], F32R, tag="wol",
                                       name="wol_sb")
                eng.dma_start(wol_sb[p][:], wol_in[p])
                if p < 3:
                    wor_sb[p] = wpool.tile([128, 2, EHALF], F32R, tag="wor",
                                           name="wor_sb")
                    eng.dma_start(wor_sb[p][:], wor_in[p])

            load_qk(0, 0, split_x=x_dma)
            load_qk(0, 1, split_x=x_dma)
            csk_sb = consts.tile([128, S], BF16, tag="cs", name="csk_sb")
            ssk_sb = consts.tile([128, S], BF16, tag="ss", name="ssk_sb")
            nc.sync.dma_start(csk_sb[:], csk_in[:])
            nc.sync.dma_start(ssk_sb[:], ssk_in[:])
            load_v(0)
            x_dma(1, 0, 4)
            bv_sb = consts.tile([128, NP, 256], BF16, tag="bv")
            nc.sync.dma_start(bv_sb[:], bv_in[:])
            triw_sb = consts.tile([128, 256], BF16, tag="triw")
            nc.sync.dma_start(triw_sb[:], triw_in[:])
            ones_sb = consts.tile([128, 128], F32R, tag="ones")
            nc.sync.dma_start(ones_sb[:], ones_in[:])
            x_dma(1, 4, 8)
            x_dma(1, 8, 12)
            x_dma(1, 12, 16)
            load_wo(0)

            # ---- DRAM tiles for the ot exchange ----
            ag_ins = []
            ag_outs = []
            for p in range(NP - 1):
                ag_ins.append(dram.tile([256, S], F32R, name=f"agi{p}"))
                ag_outs.append(dram.tile([4, 128, S], F32R, name=f"ago{p}"))
            ag3_ins = [dram.tile([256, 512], F32R, name=f"agi3q{qc}")
                       for qc in range(2)]
            ag3_outs = [dram.tile([4, 128, 512], F32R, name=f"ago3q{qc}")
                        for qc in range(2)]

            y_acc = yap.tile([128, 8, 2, 512], BF16, name="y_acc")
            ots = {}  # (p, lh, qc) -> SBUF tile [128, 512] f32r
            sotr = {}  # p -> [128, 2, S] (p<3) / (3, qc) -> [128, 2, 512]

            def rope_chunk(pm, b_ap, cs_ap, ss_ap, dst_ap):
                """dst = rope(pm + bias); all [128, 512]."""
                tmp = rope.tile([128, 512], BF16, tag="tmp", name="tmp")
                nc.scalar.activation(
                    tmp[:], pm, mybir.ActivationFunctionType.Identity, bias=b_ap
                )
                tsw = rope.tile([128, 512], BF16, tag="tsw", name="tsw")
                nc.scalar.dma_start(tsw[0:64, :], tmp[64:128, :])
                nc.scalar.dma_start(tsw[64:128, :], tmp[0:64, :])
                nc.vector.tensor_mul(tmp[:], tmp[:], cs_ap)
                nc.vector.tensor_mul(tsw[:], tsw[:], ss_ap)
                nc.vector.tensor_add(dst_ap, tmp[:], tsw[:])

            def qk_proj(p):
                q_sb = [qkv.tile([128, S], F32R, tag=f"q{lh}", name="q_sb")
                        for lh in range(2)]
                k_sb = [qkv.tile([128, S], F32R, tag=f"k{lh}", name="k_sb")
                        for lh in range(2)]
                for lh in range(2):
                    h = 2 * p + lh
                    # c-major so the startup pair consumes x in arrival order
                    for c in range(2):
                        ccols = slice(c * 512, c * 512 + 512)
                        for dst, w_sb, b_sb in (
                            (q_sb[lh], wq_sb[(p, lh)], bq_sb),
                            (k_sb[lh], wk_sb[(p, lh)], bk_sb),
                        ):
                            pm = ps_qk.tile([128, 512], F32, tag="qk", name="pm")
                            for kc in range(KC):
                                nc.tensor.matmul(pm[:], w_sb[:, kc, :],
                                                 x_sb[:, kc, ccols],
                                                 start=(kc == 0),
                                                 stop=(kc == KC - 1))
                            rope_chunk(pm[:], b_sb[:, h, :], csk_sb[:, ccols],
                                       ssk_sb[:, ccols], dst[:, ccols])
                return q_sb, k_sb

            def v_proj(p):
                v_sb = qkv.tile([128, 8, 256], F32R, tag="v", name="v_sb")
                for st in range(8):
                    scols = slice(st * 128, st * 128 + 128)
                    vp = ps_qk.tile([128, 512], F32, tag="qk", name="vp")
                    for kc in range(KC):
                        nc.tensor.matmul(vp[:, 0:256], x_sb[:, kc, scols],
                                         wv_sb[p][:, kc, :],
                                         start=(kc == 0), stop=(kc == KC - 1))
                    nc.vector.tensor_add(v_sb[:, st, :], vp[:, 0:256],
                                         bv_sb[:, p, :])
                return v_sb

            def attn_block(p, lh, qc, q_sb, k_sb, v_sb):
                """One (lh, qc) attention block, software-pipelined issue.

                Narrow 128-wide tips are widened to 256 (fp32r fast path);
                the extra columns are fully-masked garbage zeroed by the
                widened mask, harmless to the l/o accumulations.
                """
                infos = []
                kts = range(4) if qc == 0 else range(8)
                for kt in kts:
                    if qc == 0:
                        qv = slice(kt * 128, 512)
                        dg = slice(kt * 128, kt * 128 + 128)
                    else:
                        qv = slice((kt - 4) * 128, 512) if kt >= 4 \
                            else slice(0, 512)
                        dg = slice((kt - 4) * 128, (kt - 4) * 128 + 128) \
                            if kt >= 4 else None
                    wide = qv.stop - qv.start == 128
                    if wide:
                        qv = slice(qv.start - 128, 512)
                        dg = qv
                    infos.append((kt, qv, dg, wide))
                n = len(infos)
                vcols = slice(lh * 128, lh * 128 + 128)
                l_ps = ps_acc.tile([128, 512], F32, tag="acc", name="l_ps")
                o_ps = ps_acc.tile([128, 512], F32, tag="acc", name="o_ps")
                PIPE = 2
                pts = {}
                for i in range(n + PIPE):
                    if i < n:
                        kt, qv, dg, wide = infos[i]
                        qg = slice(qc * 512 + qv.start, qc * 512 + 512)
                        st_ps = ps_st.tile([128, 512], F32, tag="st",
                                           name="st_ps")
                        nc.tensor.matmul(st_ps[:, qv],
                                         k_sb[lh][:, kt * 128:kt * 128 + 128],
                                         q_sb[lh][:, qg], start=True, stop=True)
                        pt = ptp.tile([128, 512], F32R, tag="pt", name="pt")
                        nc.scalar.activation(pt[:, qv], st_ps[:, qv],
                                             mybir.ActivationFunctionType.Exp,
                                             scale=SCALE)
                        if dg is not None:
                            msk = triw_sb[:] if wide else triw_sb[:, 128:256]
                            nc.vector.tensor_mul(pt[:, dg], pt[:, dg], msk)
                        pts[i] = (pt, qv)
                    j = i - PIPE
                    if 0 <= j < n:
                        pt, qv = pts.pop(j)
                        ktj = infos[j][0]
                        first, last = (j == 0), (j == n - 1)
                        nc.tensor.matmul(l_ps[:, qv], ones_sb[:], pt[:, qv],
                                         start=first, stop=last)
                        nc.tensor.matmul(o_ps[:, qv], v_sb[:, ktj, vcols],
                                         pt[:, qv], start=first, stop=last)
                rl = ysp.tile([128, 512], F32, tag="rl", bufs=1, name="rl")
                nc.vector.reciprocal_approx_fast(rl[:], l_ps[:])
                ot = otsp.tile([128, 512], F32R, tag=f"ot{lh}{qc}", name="ot")
                nc.vector.tensor_mul(ot[:], o_ps[:], rl[:])
                ots[(p, lh, qc)] = ot
                # exchange input on the Act queue
                if p < 3:
                    nc.scalar.dma_start(
                        ag_ins[p][lh * 128:lh * 128 + 128,
                                  qc * 512:qc * 512 + 512], ot[:])
                else:
                    nc.scalar.dma_start(
                        ag3_ins[qc][lh * 128:lh * 128 + 128, :], ot[:])

            def fetch_remote(p, qc=None, eng=None):
                """Conditional DMA of the peer head-group's ot rows to SBUF."""
                if p < 3:
                    w, tag = S, "sotr"
                    src = ag_outs[p].rearrange("a r c -> r a c")
                else:
                    w, tag = 512, "sotr3"
                    src = ag3_outs[qc].rearrange("a r c -> r a c")
                t = sotp.tile([128, 2, w], F32R, tag=tag, name="sotr_t")
                if eng is None:
                    nc.scalar.dma_start(t[:], src[:, 0:2, :], cond=sel_lo)
                    nc.scalar.dma_start(t[:], src[:, 2:4, :], cond=sel_hi)
                else:
                    eng.dma_start(t[:], src[:, 0:2, :], cond=sel_lo_g)
                    eng.dma_start(t[:], src[:, 2:4, :], cond=sel_hi_g)
                sotr[p if p < 3 else (3, qc)] = t

            def o_comb(p):
                """Accumulate local pair-p + remote pair-(p-1) into y_acc."""
                for qt in range(8):
                    qc, col = qt // 4, (qt % 4) * 128
                    for eb in range(2):
                        ecols = slice(eb * 512, eb * 512 + 512)
                        pool = ps_qk if (qt * 2 + eb) % 2 == 0 else ps_st
                        tag = "qk" if (qt * 2 + eb) % 2 == 0 else "st"
                        y_ps = pool.tile([128, 512], F32, tag=tag,
                                         name="y_ps")
                        nmm = 2 if p == 0 else 4
                        mm = 0
                        for a in range(2):
                            mm += 1
                            nc.tensor.matmul(
                                y_ps[:], ots[(p, a, qc)][:, col:col + 128],
                                wol_sb[p][:, a, ecols],
                                start=(mm == 1), stop=(mm == nmm))
                        if p > 0:
                            for a in range(2):
                                mm += 1
                                nc.tensor.matmul(
                                    y_ps[:],
                                    sotr[p - 1][:, a, qt * 128:qt * 128 + 128],
                                    wor_sb[p - 1][:, a, ecols],
                                    start=False, stop=(mm == nmm))
                        ya = y_acc[:, qt, eb, :]
                        if p == 0:
                            nc.vector.tensor_copy(ya, y_ps[:])
                        else:
                            nc.vector.tensor_add(ya, ya, y_ps[:])

            def o_final(qts, sot3_ap):
                """Remote pair-3 chunks + final add and store.

                y_sb cycles through 3 borrowed buffers (pt x2 + rl) so the
                y-store DMA completion latency pipelines instead of
                serializing; y_ps alternates the qk/st PSUM pools for
                rotation slack."""
                for qt in qts:
                    col = (qt % 4) * 128
                    qcols = slice(qt * 128, qt * 128 + 128)
                    for eb in range(2):
                        ecols = slice(eb * 512, eb * 512 + 512)
                        pool = ps_qk if (qt * 2 + eb) % 2 == 0 else ps_st
                        tag = "qk" if (qt * 2 + eb) % 2 == 0 else "st"
                        y_ps = pool.tile([128, 512], F32, tag=tag,
                                         name="y_ps")
                        for a in range(2):
                            nc.tensor.matmul(
                                y_ps[:], sot3_ap[:, a, col:col + 128],
                                wor3[:, a, ecols],
                                start=(a == 0), stop=(a == 1))
                        r = (qt * 2 + eb) % 3
                        if r < 2:
                            y_sb = ptp.tile([128, 512], F32, tag="pt",
                                            name="y_sb")
                        else:
                            y_sb = ysp.tile([128, 512], F32, tag="rl",
                                            bufs=1, name="y_sb")
                        nc.vector.tensor_add(y_sb[:], y_acc[:, qt, eb, :],
                                             y_ps[:])
                        nc.scalar.dma_start(y_out[qcols, ecols], y_sb[:])

            # ================= main pair loop =================
            q_sb = k_sb = v_sb = None
            for p in range(NP):
                if p == 0:
                    q_sb, k_sb = qk_proj(0)
                    v_sb = v_proj(0)
                if p + 1 < NP:
                    load_qk(p + 1, 0)
                    load_qk(p + 1, 1)
                    load_v(p + 1)
                if p == 3:
                    # x_sb is dead after pair-3 projections; reuse its buffer
                    # for the tail tensors so no rotation wait serializes the
                    # tail (wor[3] behind o_comb(3) cost ~10us before)
                    x2 = xpool.tile([128, KC, S], F32R, tag="x", name="x2")
                    wor3 = x2[:, 0:2, :]
                    sot3 = [x2[:, 2:4, 0:512], x2[:, 4:6, 0:512]]

                # ---- attention ----
                lq = [(lh, qc) for lh in range(2) for qc in range(2)] \
                    if p < 3 else [(lh, qc) for qc in range(2) for lh in range(2)]
                for bi, (lh, qc) in enumerate(lq):
                    attn_block(p, lh, qc, q_sb, k_sb, v_sb)
                    # fetch the previous pair's peer rows mid-attention: its
                    # AllGather finished long ago, so the act queue never
                    # blocks, and the data lands well before o_comb(p)
                    if bi == 0 and p > 0:
                        fetch_remote(p - 1)
                        if p < 3:
                            load_wo(p, eng=nc.scalar)
                        else:
                            nc.scalar.dma_start(wor3, wor_in[3])
                            load_wo(3, eng=nc.scalar)
                    if p == 3 and lh == 1:
                        nc.gpsimd.collective_compute(
                            "AllGather", mybir.AluOpType.bypass,
                            replica_groups=PAIRS,
                            ins=[ag3_ins[qc].opt()],
                            outs=[ag3_outs[qc].opt()],
                        )
                if p < 3:
                    nc.gpsimd.collective_compute(
                        "AllGather", mybir.AluOpType.bypass, replica_groups=PAIRS,
                        ins=[ag_ins[p].opt()], outs=[ag_outs[p].opt()],
                    )

                # ---- next proj, then o accumulation, then V ----
                if p + 1 < NP:
                    nq, nk = qk_proj(p + 1)
                    o_comb(p)
                    nv = v_proj(p + 1)
                    q_sb, k_sb, v_sb = nq, nk, nv
                else:
                    # fetches into x2 scratch right after attention: AG3a is
                    # done, AG3b completes while o_comb(3) runs on the PE
                    for qc in range(2):
                        s3 = ag3_outs[qc].rearrange("a r c -> r a c")
                        nc.scalar.dma_start(sot3[qc], s3[:, 0:2, :],
                                            cond=sel_lo)
                        nc.scalar.dma_start(sot3[qc], s3[:, 2:4, :],
                                            cond=sel_hi)
                    o_comb(3)
                    o_final(range(0, 4), sot3[0])
                    o_final(range(4, 8), sot3[1])
    nc.compile()
    return nc


def _get_nc():
    if "nc" not in _cache:
        _cache["nc"] = _build_nc()
    return _cache["nc"]


def _head_perm():
    p = []
    for h in range(H):
        base = h * HD
        p += [base + 2 * j for j in range(HD // 2)]
        p += [base + 2 * j + 1 for j in range(HD // 2)]
    return np.array(p)


def _pack_thin(wT):
    # [2048(k), 2048(d)] -> [H, 128(p), KC, 128(d)]
    return np.ascontiguousarray(
        wT.reshape(KC, 128, H, 128).transpose(2, 1, 0, 3)
    )


def _pack_x(xb):
    # [rows 1024, 2048] -> [2(col-half), 128(p), KC, 512] (contiguous halves)
    xt = xb.T.reshape(KC, 128, 2, 512)
    return np.ascontiguousarray(xt.transpose(2, 1, 0, 3))


def _prep_inputs(inputs):
    import ml_dtypes

    BF = ml_dtypes.bfloat16
    x = np.asarray(inputs["x"], np.float32)
    freqs_cos = np.asarray(inputs["freqs_cos"], np.float32)
    freqs_sin = np.asarray(inputs["freqs_sin"], np.float32)
    mask = np.asarray(inputs["mask"], np.float32)
    wq = np.asarray(inputs["wq"], np.float32)
    bq = np.asarray(inputs["bq"], np.float32)
    wk = np.asarray(inputs["wk"], np.float32)
    bk = np.asarray(inputs["bk"], np.float32)
    wv = np.asarray(inputs["wv"], np.float32)
    bv = np.asarray(inputs["bv"], np.float32)
    wo = np.asarray(inputs["wo"], np.float32)
    start_pos = int(np.asarray(inputs.get("start_pos", 0)))

    perm = _head_perm()
    wq_all = _pack_thin(np.ascontiguousarray(wq[perm].T))  # [H,128,KC,128]
    wk_all = _pack_thin(np.ascontiguousarray(wk[perm].T))
    bq_all = bq[perm].reshape(H, 128)  # [H, 128]
    bk_all = bk[perm].reshape(H, 128)

    wvT = np.ascontiguousarray(wv.T)  # [in 2048, out 2048]
    woT = np.ascontiguousarray(wo.T)  # [d 2048, e 2048]

    cosT = freqs_cos[start_pos:start_pos + S].T.astype(np.float32)  # [64, S]
    sinT = freqs_sin[start_pos:start_pos + S].T.astype(np.float32)
    csk2 = np.ascontiguousarray(np.vstack([cosT, cosT])).astype(BF)
    ssk2 = np.ascontiguousarray(np.vstack([-sinT, sinT])).astype(BF)

    m2 = mask[0, 0]  # [S(q), S(k)] additive
    # widened tip mask [128(k), 256(q)]: first 128 q-cols all-invalid (zeros),
    # last 128 = diagonal tri block (identical for every causal diag block)
    tri_d = np.exp(m2[7 * 128:8 * 128, 7 * 128:8 * 128]).T.astype(np.float32)
    triw = np.concatenate([np.zeros((128, 128), np.float32), tri_d],
                          axis=1).astype(BF)
    ones128 = np.ones((128, 128), np.float32)

    in_maps = []
    for c in range(N_CORES):
        b, g = c // 2, c % 2
        gp = 1 - g
        hs = slice(HPC * g, HPC * g + HPC)
        wv_pre = np.ascontiguousarray(
            wvT[:, g * 1024:(g + 1) * 1024]
            .reshape(KC, 128, NP, 256).transpose(2, 1, 0, 3))
        woh = woT[:, g * 1024:(g + 1) * 1024].reshape(KC, 128, EHALF)
        wol_pre = np.ascontiguousarray(np.stack(
            [np.stack([woh[8 * g + 2 * p], woh[8 * g + 2 * p + 1]], axis=1)
             for p in range(NP)], axis=0))
        wor_pre = np.ascontiguousarray(np.stack(
            [np.stack([woh[8 * gp + 2 * p], woh[8 * gp + 2 * p + 1]], axis=1)
             for p in range(NP)], axis=0))
        bq_p = np.ascontiguousarray(bq_all[hs].T)[:, :, None]  # [128, HPC, 1]
        bk_p = np.ascontiguousarray(bk_all[hs].T)[:, :, None]
        bv_p = np.ascontiguousarray(np.broadcast_to(
            bv[g * 1024:(g + 1) * 1024].reshape(NP, 256)[None],
            (128, NP, 256))).astype(BF)
        in_maps.append({
            "x_pre": _pack_x(x[b]),
            "wq_pre": np.ascontiguousarray(wq_all[hs]),
            "wk_pre": np.ascontiguousarray(wk_all[hs]),
            "wv_pre": wv_pre,
            "wol_pre": wol_pre,
            "wor_pre": wor_pre,
            "bq_p": bq_p, "bk_p": bk_p, "bv_p": bv_p,
            "csk2": csk2, "ssk2": ssk2,
            "triw": triw, "ones128": ones128,
            "sel_lo": np.array([[1 if g == 1 else 0]], np.uint32),
            "sel_hi": np.array([[1 if g == 0 else 0]], np.uint32),
        })
    return in_maps


def kernel(**inputs):
    from concourse.bass_utils import run_bass_kernel_spmd

    trace = bool(inputs.pop("_trace", False))
    bo = np.asarray(inputs["bo"], np.float32)
    in_maps = _prep_inputs(inputs)

    nc = _get_nc()
    kwargs = {}
    if trace:
        kwargs = {"trace": True, "trace_cores": list(range(N_CORES))}
    res = run_bass_kernel_spmd(nc, in_maps, core_ids=list(range(N_CORES)), **kwargs)
    _cache["last_result"] = res

    out = np.empty((B, S, DIM), np.float32)
    for c in range(N_CORES):
        b, g = c // 2, c % 2
        out[b, :, g * 1024:(g + 1) * 1024] = (
            res.results[c]["y"] + bo[None, g * 1024:(g + 1) * 1024])
    return out
